# revision 12
# baseline (speedup 1.0000x reference)
"""DitLinearTemporalSelfAttention on 8 TRN2 NeuronCores (Bass/Tile).

Sharding: token-parallel. Core c handles batch b=c//2, token half c%2
(2048 tokens, full D=1024). The temporal-softmax/context reduction over
T=4096 spans two cores per batch -> pairwise AllReduce [[0,1],[2,3],...]
of the tiny per-batch [H,dh,dh+1] context+ksum buffer (266 KB).

Math (per core, tokens t in its slice):
  xn   = LN(x) with norm_g/norm_b folded into the weights host-side
  kT/vv: layout-A projections out[t,j] (bf16, fp32 psum), exp fused in epilogue
  ctx_unnorm[h,d,l] = sum_t expk[t,d] * v[t,l];  ksum via ones-column of v
  (pairwise AllReduce) -> ctx = ctx_unnorm / ksum; block-diag ctx_bd per
  j-chunk with ones cols -> y[t,:]+qden via ONE matmul per (tile, chunk)
  h = silu(LN(y)*scale2 + shift2);  out = x + (h @ wo_gated) (gate folded
  into out_W host-side)

Scheduling notes vs the original baseline (478us):
  - scalar engine keeps a single activation table per phase: SQRTs are
    batched per 4-tile group (table loads were 2/tile = 41us), Identity/
    Copy don't touch the table.
  - all weight DMAs issued up front (wkv first); emb MLP matmuls emitted
    after the kv loop so they don't block the PE queue.
  - em_y uses one [128,130] matmul per (tile, j-chunk) against a
    block-diagonal ctx (8 LDWEIGHTS/tile instead of 16).
  - gate folded into out_W host-side: epilogue = psum + x (2 DVE ops).
"""

import numpy as np

import concourse.bass as bass
import concourse.bacc as bacc
import concourse.mybir as mybir
import concourse.tile as tile
from concourse import masks
from concourse.bass_utils import run_bass_kernel_spmd

B, T, D, H, DH = 4, 4096, 1024, 16, 64
NCORES = 8
TL = T // 2          # tokens per core
NT = TL // 128       # 16 token tiles
ND = D // 128        # 8 d-chunks
EPS = 1e-5
FP32 = mybir.dt.float32
FP32R = mybir.dt.float32r
BF16 = mybir.dt.bfloat16

_CACHE = {}
USE_COLLECTIVE = True
USE_NATIVE_SILU = True


def r32(ap):
    return ap.bitcast(FP32R)


def _legalize_waits(nc, cap=2, escap=2):
    """Split >cap semaphore waits off any instruction into EventSemaphore
    instructions placed immediately before it on the same engine (walrus
    codegen structs hold only a few sync-wait slots)."""
    n = 0
    for bb in nc.main_func.blocks:
        out = []
        changed = False
        for ins in bb.instructions:
            si = ins.sync_info
            ty = type(ins).__name__
            icap = 1 if ty == "InstDMACopy" else cap
            if (si is not None and si.on_wait is not None
                    and len(si.on_wait) > icap
                    and ty not in ("InstDrain", "InstEventSemaphore")):
                waits = list(si.on_wait)
                keep, extra = waits[:icap], waits[icap:]
                while extra:
                    chunk, extra = extra[:escap], extra[escap:]
                    n += 1
                    es = mybir.InstEventSemaphore(
                        name=f"I-wsplit-{n}", engine=ins.engine,
                        sync_info=mybir.SyncInfo(on_wait=list(chunk),
                                                 on_update=[]))
                    out.append(es)
                ins.sync_info = mybir.SyncInfo(
                    on_wait=keep, on_update=list(si.on_update or []))
                changed = True
            out.append(ins)
        if changed:
            bb.instructions = out
    return n


def build(has_cq, has_ck, has_cv, has_co, has_cemb):
    from contextlib import ExitStack

    nc = bacc.Bacc("TRN2", target_bir_lowering=False, debug=False,
                   num_devices=NCORES)

    x_d = nc.dram_tensor("x", [TL, D], BF16, kind="ExternalInput")
    wkv_d = nc.dram_tensor("wkv", [2 * D, D], BF16, kind="ExternalInput")
    wq_d = nc.dram_tensor("wq", [D, D], BF16, kind="ExternalInput")
    wo_d = nc.dram_tensor("wo", [D, D], BF16, kind="ExternalInput")
    embw_d = nc.dram_tensor("embw", [D, 2 * D], BF16, kind="ExternalInput")
    embt_d = nc.dram_tensor("embt", [D], FP32, kind="ExternalInput")
    gsn_d = nc.dram_tensor("gsn", [2, D], FP32R, kind="ExternalInput")
    cemb_d = nc.dram_tensor("cemb", [2 * D], FP32R, kind="ExternalInput") if has_cemb else None
    cq_d = nc.dram_tensor("cq", [D], FP32R, kind="ExternalInput") if has_cq else None
    ck_d = nc.dram_tensor("ck", [D], FP32R, kind="ExternalInput") if has_ck else None
    cv_d = nc.dram_tensor("cv", [D], FP32R, kind="ExternalInput") if has_cv else None
    co_d = nc.dram_tensor("co", [D], FP32R, kind="ExternalInput") if has_co else None
    out_d = nc.dram_tensor("out", [TL, D], FP32, kind="ExternalOutput")

    def _emit(tc, es):
        constp = es.enter_context(tc.tile_pool(name="const", bufs=1))
        xio = es.enter_context(tc.tile_pool(name="xio", bufs=6))
        finp = es.enter_context(tc.tile_pool(name="finp", bufs=2))
        xnp = es.enter_context(tc.tile_pool(name="xnp", bufs=3))
        statp = es.enter_context(tc.tile_pool(name="stat", bufs=4))
        dramp = es.enter_context(tc.tile_pool(name="dram", bufs=1, space="DRAM"))
        tp = es.enter_context(tc.tile_pool(name="tp", bufs=2, space="PSUM"))
        pp = es.enter_context(tc.tile_pool(name="pp", bufs=6, space="PSUM"))

        # ---------------- constants ----------------
        ident = constp.tile([128, 128], FP32)
        masks.make_identity(nc, ident[:])
        ones_row32 = constp.tile([1, 512], FP32)
        nc.vector.memset(ones_row32[:], 1.0)
        ones_row = constp.tile([1, 512], FP32R)
        nc.vector.tensor_copy(ones_row[:], ones_row32[:])

        # xnT opens BEFORE setup transients so it never reuses their zone
        es_xnt = ExitStack()
        xntp = es_xnt.enter_context(tc.tile_pool(name="xnT", bufs=1))
        xnt = xntp.tile([128, ND * TL], BF16)

        es_wq = ExitStack()
        wqp = es_wq.enter_context(tc.tile_pool(name="wq", bufs=1))
        wq = wqp.tile([128, ND * 1024], BF16)

        es_wkv = ExitStack()
        wkvp = es_wkv.enter_context(tc.tile_pool(name="wkv", bufs=1))
        kvp = es_wkv.enter_context(tc.tile_pool(name="kv", bufs=2))
        wkv = wkvp.tile([128, 2 * ND * 1024], BF16)
        # wkv DMA first: the kv loop needs it earliest
        nc.sync.dma_start(
            out=wkv[:].rearrange("p (dc c) -> p dc c", c=D),
            in_=wkv_d[:].rearrange("(dc p) c -> p dc c", p=128))
        nc.sync.dma_start(
            out=wq[:].rearrange("p (dc c) -> p dc c", c=D),
            in_=wq_d[:].rearrange("(dc p) c -> p dc c", p=128))

        es_rows = ExitStack()
        rowsp = es_rows.enter_context(tc.tile_pool(name="rows", bufs=1))
        embp = es_rows.enter_context(tc.tile_pool(name="embp", bufs=2))

        # rows: sng/snb via ONE dma; bias rows when present
        gsn = rowsp.tile([1, 2 * D], FP32R)
        nc.sync.dma_start(out=gsn[:], in_=gsn_d[:].rearrange("a b -> (a b)").unsqueeze(0))
        sng_row = gsn[:, 0:D]
        snb_row = gsn[:, D:2 * D]

        def load_row(pool, dram_ap, n):
            t_ = pool.tile([1, n], FP32R, tag=dram_ap.tensor.name)
            nc.sync.dma_start(out=t_[:], in_=dram_ap.unsqueeze(0))
            return t_

        cemb_row = load_row(constp, cemb_d.ap(), 2 * D) if has_cemb else None
        cq_row = load_row(constp, cq_d.ap(), D) if has_cq else None
        ck_row = load_row(constp, ck_d.ap(), D) if has_ck else None
        cv_row = load_row(constp, cv_d.ap(), D) if has_cv else None
        co_row = load_row(constp, co_d.ap(), D) if has_co else None

        # embt host-permuted: one DMA fills [128, 8], (p, c) = emb[c*128+p]
        embt_sb = rowsp.tile([128, ND], FP32)
        nc.sync.dma_start(out=embt_sb[:], in_=embt_d[:].rearrange(
            "(p c) -> p c", c=ND))

        # ---------------- stage 0+1: load x, LN (batched sqrt), kv ----------------
        # per-group LN stat tiles (group = 4 token tiles)
        aggs, rstds, nmrs = {}, {}, {}

        def em_load(t):
            xt = xio.tile([128, D], BF16, tag="xin")
            nc.sync.dma_start(out=xt[:], in_=x_d[t * 128:(t + 1) * 128, :])
            return xt

        xts = {}

        def em_stats(t):
            xt = xts[t]
            st6 = statp.tile([128, 2, 6], FP32, tag="st6")
            nc.vector.bn_stats(st6[:, 0, :], xt[:, 0:512])
            nc.vector.bn_stats(st6[:, 1, :], xt[:, 512:1024])
            g, i = t // 4, t % 4
            if i == 0:
                agg4 = statp.tile([128, 2, 4], FP32, tag="agg4")
                aggs[g] = agg4
            nc.vector.bn_aggr(aggs[g][:, :, i], st6[:])

        def em_group(g):
            # rstd = 1/sqrt(var+eps), nmr = -mean*rstd for 4 tiles at once
            agg = aggs.pop(g)
            rstd = statp.tile([128, 4], FP32, tag="rstd4")
            nc.vector.tensor_scalar(rstd[:], agg[:, 1, :], EPS, None,
                                    mybir.AluOpType.add)
            nc.scalar.sqrt(rstd[:], rstd[:])
            nc.vector.reciprocal(rstd[:], rstd[:])
            nmr = statp.tile([128, 4], FP32, tag="nmr4")
            nc.vector.scalar_tensor_tensor(nmr[:], agg[:, 0, :], -1.0,
                                           rstd[:], mybir.AluOpType.mult,
                                           mybir.AluOpType.mult)
            rstds[g], nmrs[g] = rstd, nmr

        def em_apply_tp(t):
            xt = xts.pop(t)
            g, i = t // 4, t % 4
            xn = xnp.tile([128, D], FP32, tag="xn")
            nc.scalar.activation(xn[:], xt[:],
                                 mybir.ActivationFunctionType.Identity,
                                 bias=nmrs[g][:, i:i + 1],
                                 scale=rstds[g][:, i:i + 1])
            for gg in range(2):  # groups of 4 d-chunks
                tpt = tp.tile([128, 512], FP32, tag="tp")
                for ii in range(4):
                    dc = gg * 4 + ii
                    nc.tensor.transpose(tpt[:, ii * 128:(ii + 1) * 128],
                                        xn[:, dc * 128:(dc + 1) * 128],
                                        ident[:])
                dst = xnt[:].rearrange("p (dc tt) -> p dc tt", tt=TL)[
                    :, gg * 4:(gg + 1) * 4, t * 128:(t + 1) * 128]
                src_ = tpt[:].rearrange("p (i c) -> p i c", c=128)
                nc.vector.tensor_copy(dst, src_)

        ctx_sb = constp.tile([128, 8 * 65], FP32)

        def em_kv(t):
            ke = kvp.tile([128, D], BF16, tag="ke")
            va = kvp.tile([128, H * 66], BF16, tag="va")
            lhss = [xnt[:, dc * TL + t * 128: dc * TL + (t + 1) * 128]
                    for dc in range(ND)]
            for jh in range(2):
                kh = pp.tile([128, 512], FP32, tag="pp")
                for dc in range(ND):
                    nc.tensor.matmul(
                        kh[:], lhss[dc],
                        wkv[:, dc * 1024 + jh * 512:dc * 1024 + (jh + 1) * 512],
                        start=(dc == 0), stop=(dc == 7 and not has_ck))
                if has_ck:
                    nc.tensor.matmul(kh[:], ones_row[:, 0:128],
                                     ck_row[:, jh * 512:(jh + 1) * 512],
                                     start=False, stop=True)
                nc.scalar.activation(ke[:, jh * 512:(jh + 1) * 512], kh[:],
                                     mybir.ActivationFunctionType.Exp)
            for jh in range(2):
                vh = pp.tile([128, 512], FP32, tag="pp")
                for dc in range(ND):
                    nc.tensor.matmul(
                        vh[:], lhss[dc],
                        wkv[:, 8192 + dc * 1024 + jh * 512:8192 + dc * 1024 + (jh + 1) * 512],
                        start=(dc == 0), stop=(dc == 7 and not has_cv))
                if has_cv:
                    nc.tensor.matmul(vh[:], ones_row[:, 0:128],
                                     cv_row[:, jh * 512:(jh + 1) * 512],
                                     start=False, stop=True)
                nc.scalar.copy(
                    va[:].rearrange("p (h l) -> p h l", l=66)[
                        :, jh * 8:(jh + 1) * 8, 0:64],
                    vh[:].rearrange("p (h l) -> p h l", l=64))
            nc.vector.memset(
                va[:].rearrange("p (h l) -> p h l", l=66)[:, :, 64:65], 1.0)
            # one single-shot matmul per psum tile (bank sharing between
            # single-shot groups corrupts data on HW), then DVE-accumulate
            # into the SBUF ctx accumulator. Heads pack pairwise by parity.
            for hp in range(H // 2):
                cp = tp.tile([128, 512], FP32, tag="tp")
                for par in range(2):
                    h = 2 * hp + par
                    nc.tensor.matmul(cp[par * 64:par * 64 + 64, 0:65],
                                     ke[:, h * 64:(h + 1) * 64],
                                     va[:, h * 66:h * 66 + 65])
                if t == 0:
                    nc.vector.tensor_copy(
                        ctx_sb[:, hp * 65:(hp + 1) * 65], cp[:, 0:65])
                else:
                    nc.vector.tensor_tensor(
                        ctx_sb[:, hp * 65:(hp + 1) * 65],
                        ctx_sb[:, hp * 65:(hp + 1) * 65],
                        cp[:, 0:65], mybir.AluOpType.add)

        for tt in range(NT + 6):
            if tt < NT:
                xts[tt] = em_load(tt)
                em_stats(tt)
                if tt % 4 == 3:
                    em_group(tt // 4)
            if 4 <= tt < NT + 4:
                em_apply_tp(tt - 4)
            if tt >= 6:
                em_kv(tt - 6)

        # ---------------- emb MLP (emitted after kv: PE queue stays dense) ----------------
        silu_e = rowsp.tile([128, ND], FP32)
        if USE_NATIVE_SILU:
            nc.scalar.activation(silu_e[:], embt_sb[:],
                                 mybir.ActivationFunctionType.Silu)
        else:
            nc.scalar.activation(silu_e[:], embt_sb[:],
                                 mybir.ActivationFunctionType.Sigmoid)
            nc.vector.tensor_tensor(silu_e[:], silu_e[:], embt_sb[:],
                                    mybir.AluOpType.mult)
        silu_eb = rowsp.tile([128, 2 * ND], BF16)
        nc.vector.tensor_copy(
            silu_eb[:].rearrange("p (c two) -> p c two", two=2)[:, :, 0:1],
            silu_e[:].unsqueeze(2))
        emb_sel = rowsp.tile([1, 2 * D], FP32R)
        for dc in range(ND):
            embw_ch = embp.tile([128, 2 * D], BF16, tag="embw_ch")
            nc.sync.dma_start(out=embw_ch[:],
                              in_=embw_d[dc * 128:(dc + 1) * 128, :])
            for nch in range(4):
                epn = pp.tile([1, 512], FP32, tag="pp")
                nc.tensor.matmul(epn[:],
                                 silu_eb[:, 2 * dc:2 * dc + 1],
                                 embw_ch[:, nch * 512:(nch + 1) * 512])
                dst = emb_sel[:, nch * 512:(nch + 1) * 512]
                if dc == 0:
                    nc.vector.tensor_copy(dst, epn[:])
                else:
                    nc.vector.tensor_tensor(dst, dst, epn[:],
                                            mybir.AluOpType.add)
        if has_cemb:
            nc.vector.tensor_tensor(emb_sel[:], emb_sel[:], cemb_row[:],
                                    mybir.AluOpType.add)
        # broadcast emb_sel + sng/snb rows to all partitions
        emb_sel_b = rowsp.tile([128, 2 * D], FP32)
        for nch in range(4):
            bp = tp.tile([128, 512], FP32, tag="tp")
            nc.tensor.matmul(bp[:], ones_row[:, 0:128],
                             emb_sel[:, nch * 512:(nch + 1) * 512])
            nc.vector.tensor_copy(emb_sel_b[:, nch * 512:(nch + 1) * 512], bp[:])

        def bcast(row, name):
            out = rowsp.tile([128, D], FP32, tag=f"bc_{name}")
            for nh in range(2):
                bp = tp.tile([128, 512], FP32, tag="tp")
                nc.tensor.matmul(bp[:], ones_row[:, 0:128],
                                 row[:, nh * 512:(nh + 1) * 512])
                nc.vector.tensor_copy(out[:, nh * 512:(nh + 1) * 512], bp[:])
            return out

        sng_b = bcast(sng_row, "sng")
        snb_b = bcast(snb_row, "snb")
        # scale2 = sng*(1+scale); shift2 = snb*(1+scale) + shift
        t1_b = rowsp.tile([128, D], FP32)
        nc.vector.tensor_scalar(t1_b[:], emb_sel_b[:, 0:D], 1.0, None,
                                mybir.AluOpType.add)
        s2_b = constp.tile([128, D], FP32)
        nc.vector.tensor_tensor(s2_b[:], t1_b[:], sng_b[:],
                                mybir.AluOpType.mult)
        sh2_b = constp.tile([128, D], FP32)
        nc.vector.tensor_tensor(sh2_b[:], t1_b[:], snb_b[:],
                                mybir.AluOpType.mult)
        nc.vector.tensor_tensor(sh2_b[:], sh2_b[:], emb_sel_b[:, D:2 * D],
                                mybir.AluOpType.add)
        es_rows.close()

        # ---------------- stage 3a: stage ctx + pairwise AllReduce ----------------
        cc_in = dramp.tile([H, 64, 65], FP32)
        cc_out = dramp.tile([H, 64, 65], FP32)
        for q in range(2):
            nc.sync.dma_start(
                out=cc_in[:].rearrange("(g q) d l -> q d g l", q=2)[q],
                in_=ctx_sb[q * 64:(q + 1) * 64, :].rearrange(
                    "d (g l) -> d g l", l=65))
        if USE_COLLECTIVE:
            nc.gpsimd.collective_compute(
                "AllReduce", mybir.AluOpType.add,
                replica_groups=[[0, 1], [2, 3], [4, 5], [6, 7]],
                ins=[cc_in.opt()], outs=[cc_out.opt()])
        else:
            nc.sync.dma_start(out=cc_out[:], in_=cc_in[:])
        es_wkv.close()

        # ---------------- stage 2: q proj + exp (overlaps collective) ----------------
        es_qt = ExitStack()
        qtp = es_qt.enter_context(tc.tile_pool(name="qT", bufs=1, side="right"))
        es_out = ExitStack()
        wop = es_out.enter_context(tc.tile_pool(name="wo", bufs=1, side="right"))
        ytp = es_out.enter_context(tc.tile_pool(name="ytp", bufs=5, side="right"))
        lnp = es_out.enter_context(tc.tile_pool(name="lnp", bufs=2, side="right"))
        hsp = es_out.enter_context(tc.tile_pool(name="hsp", bufs=3, side="right"))
        wo = wop.tile([128, ND * 1024], BF16)
        nc.sync.dma_start(
            out=wo[:].rearrange("p (dc c) -> p dc c", c=D),
            in_=wo_d[:].rearrange("(dc p) c -> p dc c", p=128))
        qT = qtp.tile([128, ND * TL], BF16)  # j-chunk jc at cols jc*TL
        for jc in range(ND):
            for th in range(TL // 512):
                qps = pp.tile([128, 512], FP32, tag="pp")
                for dc in range(ND):
                    nc.tensor.matmul(
                        qps[:],
                        wq[:, dc * 1024 + jc * 128:dc * 1024 + jc * 128 + 128],
                        xnt[:, dc * TL + th * 512:dc * TL + (th + 1) * 512],
                        start=(dc == 0), stop=(dc == 7 and not has_cq))
                if has_cq:
                    nc.tensor.matmul(qps[:],
                                     cq_row[:, jc * 128:(jc + 1) * 128],
                                     ones_row[:], start=False, stop=True)
                nc.scalar.activation(
                    qT[:, jc * TL + th * 512:jc * TL + (th + 1) * 512],
                    qps[:], mybir.ActivationFunctionType.Exp)
        es_wq.close()
        es_xnt.close()

        # ---------------- stage 4: readback + normalize ctx into block-diag ----------------
        # ctx_bd layout per j-chunk jc (130 bf16 cols at jc*130):
        #   cols 0:64    = ctx_hat for head 2jc   on partitions 0:64, zeros below
        #   cols 64:128  = ctx_hat for head 2jc+1 on partitions 64:128, zeros above
        #   col 128      = ones on partitions 0:64   (-> qden of head 2jc)
        #   col 129      = ones on partitions 64:128 (-> qden of head 2jc+1)
        ctxn = constp.tile([128, 8 * 65], FP32)
        for q in range(2):
            nc.sync.dma_start(
                out=ctxn[q * 64:(q + 1) * 64, :].rearrange(
                    "d (g l) -> d g l", l=65),
                in_=cc_out[:].rearrange("(g q) d l -> q d g l", q=2)[q])
        rk = statp.tile([128, 8], FP32, tag="rk")
        nc.vector.reciprocal(
            rk[:], ctxn[:].rearrange("p (g l) -> p g l", l=65)[:, :, 64])
        ctx_bd = constp.tile([128, 8 * 130], BF16)
        nc.vector.memset(ctx_bd[:], 0.0)
        for g in range(8):
            for par in range(2):
                nc.vector.tensor_scalar(
                    ctx_bd[par * 64:(par + 1) * 64,
                           g * 130 + par * 64:g * 130 + par * 64 + 64],
                    ctxn[par * 64:(par + 1) * 64, g * 65:g * 65 + 64],
                    rk[par * 64:(par + 1) * 64, g:g + 1], None,
                    mybir.AluOpType.mult)
            nc.vector.memset(ctx_bd[0:64, g * 130 + 128:g * 130 + 129], 1.0)
            nc.vector.memset(ctx_bd[64:128, g * 130 + 129:g * 130 + 130], 1.0)

        # ---------------- stage 5+6 fused per token tile ----------------
        yts, hss = {}, {}
        agg2s, rstd2s, nmr2s = {}, {}, {}

        def em_y(t):
            yt = ytp.tile([128, D], FP32, tag="yt")
            yts[t] = yt
            for jc in range(ND):
                ypf = pp.tile([128, 512], FP32, tag="pp")
                yp = ypf[:, 0:130]
                nc.tensor.matmul(
                    yp[:],
                    qT[:, jc * TL + t * 128:jc * TL + (t + 1) * 128],
                    ctx_bd[:, jc * 130:(jc + 1) * 130])
                rq = statp.tile([128, 2], FP32, tag="rq")
                nc.vector.reciprocal(rq[:], yp[:, 128:130])
                # y[:, jc*128 : (jc+1)*128] = yp[:, 0:128] * qden^-1 per head
                nc.scalar.mul(yt[:, jc * 128:jc * 128 + 64], yp[:, 0:64],
                              rq[:, 0:1])
                nc.vector.tensor_scalar(yt[:, jc * 128 + 64:jc * 128 + 128],
                                        yp[:, 64:128], rq[:, 1:2],
                                        None, mybir.AluOpType.mult)

        def em_stats2(t):
            yt = yts[t]
            st6 = statp.tile([128, 2, 6], FP32, tag="st6")
            nc.vector.bn_stats(st6[:, 0, :], yt[:, 0:512])
            nc.vector.bn_stats(st6[:, 1, :], yt[:, 512:1024])
            g, i = t // 4, t % 4
            if i == 0:
                agg4b = statp.tile([128, 2, 4], FP32, tag="agg4b")
                agg2s[g] = agg4b
            nc.vector.bn_aggr(agg2s[g][:, :, i], st6[:])

        def em_group2(g):
            agg = agg2s.pop(g)
            rstd = statp.tile([128, 4], FP32, tag="rstd4b")
            nc.vector.tensor_scalar(rstd[:], agg[:, 1, :], EPS, None,
                                    mybir.AluOpType.add)
            nc.scalar.sqrt(rstd[:], rstd[:])
            nc.vector.reciprocal(rstd[:], rstd[:])
            nmr = statp.tile([128, 4], FP32, tag="nmr4b")
            nc.vector.scalar_tensor_tensor(nmr[:], agg[:, 0, :], -1.0,
                                           rstd[:], mybir.AluOpType.mult,
                                           mybir.AluOpType.mult)
            rstd2s[g], nmr2s[g] = rstd, nmr

        def em_ln(t):
            yt = yts.pop(t)
            g, i = t // 4, t % 4
            ln = lnp.tile([128, D], FP32, tag="ln")
            nc.scalar.activation(ln[:], yt[:],
                                 mybir.ActivationFunctionType.Identity,
                                 bias=nmr2s[g][:, i:i + 1],
                                 scale=rstd2s[g][:, i:i + 1])
            nc.vector.tensor_tensor(ln[:], ln[:], s2_b[:],
                                    mybir.AluOpType.mult)
            nc.gpsimd.tensor_tensor(ln[:], ln[:], sh2_b[:],
                                    mybir.AluOpType.add)
            hs = hsp.tile([128, D], FP32, tag="hs")
            if USE_NATIVE_SILU:
                nc.scalar.activation(hs[:], ln[:],
                                     mybir.ActivationFunctionType.Silu)
            else:
                nc.scalar.activation(hs[:], ln[:],
                                     mybir.ActivationFunctionType.Sigmoid)
                nc.vector.tensor_tensor(hs[:], hs[:], ln[:],
                                        mybir.AluOpType.mult)
            hss[t] = hs

        def em_out(t):
            hs = hss.pop(t)
            hst = hsp.tile([128, D], BF16, tag="hst")
            for g in range(2):
                tpt = tp.tile([128, 512], FP32, tag="tp")
                for i in range(4):
                    dc = g * 4 + i
                    nc.tensor.transpose(tpt[:, i * 128:(i + 1) * 128],
                                        hs[:, dc * 128:(dc + 1) * 128],
                                        ident[:])
                nc.vector.tensor_copy(hst[:, g * 512:(g + 1) * 512], tpt[:])
            xt2 = xio.tile([128, D], BF16, tag="xin")
            nc.gpsimd.dma_start(out=xt2[:], in_=x_d[t * 128:(t + 1) * 128, :])
            fin = finp.tile([128, D], FP32, tag="fin")
            for jh in range(2):
                oph = pp.tile([128, 512], FP32, tag="pp")
                for dc in range(ND):
                    nc.tensor.matmul(
                        oph[:], hst[:, dc * 128:(dc + 1) * 128],
                        wo[:, dc * 1024 + jh * 512:dc * 1024 + (jh + 1) * 512],
                        start=(dc == 0), stop=(dc == 7 and not has_co))
                if has_co:
                    nc.tensor.matmul(oph[:], ones_row[:, 0:128],
                                     co_row[:, jh * 512:(jh + 1) * 512],
                                     start=False, stop=True)
                nc.vector.tensor_tensor(fin[:, jh * 512:(jh + 1) * 512],
                                        oph[:],
                                        xt2[:, jh * 512:(jh + 1) * 512],
                                        mybir.AluOpType.add)
            nc.sync.dma_start(out=out_d[t * 128:(t + 1) * 128, :], in_=fin[:])

        for tt in range(NT + 6):
            if tt < NT:
                em_y(tt)
                em_stats2(tt)
                if tt % 4 == 3:
                    em_group2(tt // 4)
            if 4 <= tt < NT + 4:
                em_ln(tt - 4)
            if tt >= 6:
                em_out(tt - 6)
        es_out.close()
        es_qt.close()

    with tile.TileContext(nc) as tc, ExitStack() as es:
        _emit(tc, es)
    nc.compile()
    _legalize_waits(nc)
    return nc


def kernel(**inputs):
    x = np.asarray(inputs["x"], np.float32)
    emb = np.asarray(inputs["emb"], np.float32)
    gate_msa = np.asarray(inputs["gate_msa"], np.float32)
    norm_g = np.asarray(inputs["norm_g"], np.float32)
    norm_b = np.asarray(inputs["norm_b"], np.float32)
    Wq = np.asarray(inputs["Wq"], np.float32)
    bq = np.asarray(inputs["bq"], np.float32)
    Wk = np.asarray(inputs["Wk"], np.float32)
    bk = np.asarray(inputs["bk"], np.float32)
    Wv = np.asarray(inputs["Wv"], np.float32)
    bv = np.asarray(inputs["bv"], np.float32)
    emb_W = np.asarray(inputs["emb_W"], np.float32)
    emb_b = np.asarray(inputs["emb_b"], np.float32)
    sn_g = np.asarray(inputs["sn_g"], np.float32)
    sn_b = np.asarray(inputs["sn_b"], np.float32)
    out_W = np.asarray(inputs["out_W"], np.float32)
    out_b = np.asarray(inputs["out_b"], np.float32)

    import ml_dtypes

    # fold layernorm affine into projection weights
    wq_f = norm_g[:, None] * Wq
    wk_f = norm_g[:, None] * Wk
    wv_f = norm_g[:, None] * Wv
    wkv_f = np.ascontiguousarray(
        np.concatenate([wk_f, wv_f], axis=0).astype(ml_dtypes.bfloat16))
    wq_f = wq_f.astype(ml_dtypes.bfloat16)
    embw_bf = emb_W.astype(ml_dtypes.bfloat16)
    cq = norm_b @ Wq + bq
    ck = norm_b @ Wk + bk
    cv = norm_b @ Wv + bv

    # gate folded into out_W per batch: out = x + (h @ (out_W*gate)) + gate*out_b
    wo_gated = [np.ascontiguousarray(
        (out_W * gate_msa[b, 0, :][None, :]).astype(ml_dtypes.bfloat16))
        for b in range(B)]
    co_gated = [gate_msa[b, 0, :] * out_b for b in range(B)]

    flags = (bool(np.any(cq)), bool(np.any(ck)), bool(np.any(cv)),
             bool(np.any(out_b)), bool(np.any(emb_b)))
    if flags not in _CACHE:
        _CACHE[flags] = build(*flags)
    nc = _CACHE[flags]

    in_maps = []
    for c in range(NCORES):
        b, half = c // 2, c % 2
        m = {
            "x": np.ascontiguousarray(
                x[b, half * TL:(half + 1) * TL, :].astype(ml_dtypes.bfloat16)),
            "wq": wq_f, "wkv": wkv_f, "wo": wo_gated[b],
            "embw": embw_bf,
            "embt": np.ascontiguousarray(emb[b, 0, :].reshape(ND, 128).T.ravel()),
            "gsn": np.ascontiguousarray(np.stack([sn_g, sn_b])),
        }
        if flags[0]:
            m["cq"] = cq
        if flags[1]:
            m["ck"] = ck
        if flags[2]:
            m["cv"] = cv
        if flags[3]:
            m["co"] = co_gated[b]
        if flags[4]:
            m["cemb"] = emb_b
        in_maps.append(m)

    res = run_bass_kernel_spmd(nc, in_maps, core_ids=list(range(NCORES)),
                               **_RUN_KW)
    kernel.last_result = res
    out = np.stack([res.results[c]["out"] for c in range(NCORES)])
    return out.reshape(B, 2, TL, D).reshape(B, T, D)


_RUN_KW = {}
kernel.last_result = None


# revision 14
# speedup vs baseline: 1.0759x; 1.0759x over previous
"""DitLinearTemporalSelfAttention on 8 TRN2 NeuronCores (Bass/Tile).

Sharding: token-parallel. Core c handles batch b=c//2, token half c%2
(2048 tokens, full D=1024). The temporal-softmax/context reduction over
T=4096 spans two cores per batch -> pairwise AllReduce [[0,1],[2,3],...]
of the tiny per-batch [H,dh,dh+1] context+ksum buffer (266 KB).

Math (per core, tokens t in its slice):
  xn   = LN(x) with norm_g/norm_b folded into the weights host-side
  kT/vv: layout-A projections out[t,j] (bf16, fp32 psum), exp fused in epilogue
  ctx_unnorm[h,d,l] = sum_t expk[t,d] * v[t,l];  ksum via ones-column of v
  (pairwise AllReduce) -> ctx = ctx_unnorm / ksum; block-diag ctx_bd per
  j-chunk with ones cols -> y[t,:]+qden via ONE matmul per (tile, chunk)
  h = silu(LN(y)*scale2 + shift2);  out = x + (h @ wo_gated) (gate folded
  into out_W host-side)

Scheduling notes vs the original baseline (478us):
  - scalar engine keeps a single activation table per phase: SQRTs are
    batched per 4-tile group (table loads were 2/tile = 41us), Identity/
    Copy don't touch the table.
  - all weight DMAs issued up front (wkv first); emb MLP matmuls emitted
    after the kv loop so they don't block the PE queue.
  - em_y uses one [128,130] matmul per (tile, j-chunk) against a
    block-diagonal ctx (8 LDWEIGHTS/tile instead of 16).
  - gate folded into out_W host-side: epilogue = psum + x (2 DVE ops).
"""

import numpy as np

import concourse.bass as bass
import concourse.bacc as bacc
import concourse.mybir as mybir
import concourse.tile as tile
from concourse import masks
from concourse.bass_utils import run_bass_kernel_spmd

B, T, D, H, DH = 4, 4096, 1024, 16, 64
NCORES = 8
TL = T // 2          # tokens per core
NT = TL // 128       # 16 token tiles
ND = D // 128        # 8 d-chunks
EPS = 1e-5
FP32 = mybir.dt.float32
FP32R = mybir.dt.float32r
BF16 = mybir.dt.bfloat16

_CACHE = {}
USE_COLLECTIVE = True
USE_NATIVE_SILU = True


def r32(ap):
    return ap.bitcast(FP32R)


def _legalize_waits(nc, cap=2, escap=2):
    """Split >cap semaphore waits off any instruction into EventSemaphore
    instructions placed immediately before it on the same engine (walrus
    codegen structs hold only a few sync-wait slots)."""
    n = 0
    for bb in nc.main_func.blocks:
        out = []
        changed = False
        for ins in bb.instructions:
            si = ins.sync_info
            ty = type(ins).__name__
            icap = 1 if ty == "InstDMACopy" else cap
            if (si is not None and si.on_wait is not None
                    and len(si.on_wait) > icap
                    and ty not in ("InstDrain", "InstEventSemaphore")):
                waits = list(si.on_wait)
                keep, extra = waits[:icap], waits[icap:]
                while extra:
                    chunk, extra = extra[:escap], extra[escap:]
                    n += 1
                    es = mybir.InstEventSemaphore(
                        name=f"I-wsplit-{n}", engine=ins.engine,
                        sync_info=mybir.SyncInfo(on_wait=list(chunk),
                                                 on_update=[]))
                    out.append(es)
                ins.sync_info = mybir.SyncInfo(
                    on_wait=keep, on_update=list(si.on_update or []))
                changed = True
            out.append(ins)
        if changed:
            bb.instructions = out
    return n


def build(has_cq, has_ck, has_cv, has_co, has_cemb):
    from contextlib import ExitStack

    nc = bacc.Bacc("TRN2", target_bir_lowering=False, debug=False,
                   num_devices=NCORES)

    x_d = nc.dram_tensor("x", [TL, D], BF16, kind="ExternalInput")
    wkv_d = nc.dram_tensor("wkv", [2 * D, D], BF16, kind="ExternalInput")
    wq_d = nc.dram_tensor("wq", [D, D], BF16, kind="ExternalInput")
    wo_d = nc.dram_tensor("wo", [D, D], BF16, kind="ExternalInput")
    embw_d = nc.dram_tensor("embw", [D, 2 * D], BF16, kind="ExternalInput")
    embt_d = nc.dram_tensor("embt", [D], FP32, kind="ExternalInput")
    gsn_d = nc.dram_tensor("gsn", [2, D], FP32R, kind="ExternalInput")
    cemb_d = nc.dram_tensor("cemb", [2 * D], FP32R, kind="ExternalInput") if has_cemb else None
    cq_d = nc.dram_tensor("cq", [D], FP32R, kind="ExternalInput") if has_cq else None
    ck_d = nc.dram_tensor("ck", [D], FP32R, kind="ExternalInput") if has_ck else None
    cv_d = nc.dram_tensor("cv", [D], FP32R, kind="ExternalInput") if has_cv else None
    co_d = nc.dram_tensor("co", [D], FP32R, kind="ExternalInput") if has_co else None
    out_d = nc.dram_tensor("out", [TL, D], FP32, kind="ExternalOutput")

    def _emit(tc, es):
        constp = es.enter_context(tc.tile_pool(name="const", bufs=1))
        xio = es.enter_context(tc.tile_pool(name="xio", bufs=6))
        finp = es.enter_context(tc.tile_pool(name="finp", bufs=2))
        xnp = es.enter_context(tc.tile_pool(name="xnp", bufs=3))
        statp = es.enter_context(tc.tile_pool(name="stat", bufs=4))
        dramp = es.enter_context(tc.tile_pool(name="dram", bufs=1, space="DRAM"))
        tp = es.enter_context(tc.tile_pool(name="tp", bufs=2, space="PSUM"))
        pp = es.enter_context(tc.tile_pool(name="pp", bufs=6, space="PSUM"))

        # ---------------- constants ----------------
        ident = constp.tile([128, 128], FP32)
        masks.make_identity(nc, ident[:])
        ident_bf = constp.tile([128, 128], BF16)
        nc.vector.tensor_copy(ident_bf[:], ident[:])
        ones_row32 = constp.tile([1, 512], FP32)
        nc.vector.memset(ones_row32[:], 1.0)
        ones_row = constp.tile([1, 512], FP32R)
        nc.vector.tensor_copy(ones_row[:], ones_row32[:])

        # xnT opens BEFORE setup transients so it never reuses their zone
        es_xnt = ExitStack()
        xntp = es_xnt.enter_context(tc.tile_pool(name="xnT", bufs=1))
        xnt = xntp.tile([128, ND * TL], BF16)

        es_wq = ExitStack()
        wqp = es_wq.enter_context(tc.tile_pool(name="wq", bufs=1))
        wq = wqp.tile([128, ND * 1024], BF16)

        es_wkv = ExitStack()
        wkvp = es_wkv.enter_context(tc.tile_pool(name="wkv", bufs=1))
        kvp = es_wkv.enter_context(tc.tile_pool(name="kv", bufs=2))
        wkv = wkvp.tile([128, 2 * ND * 1024], BF16)
        # weight DMAs ride the gpsimd ring so x tiles own the sync ring
        nc.gpsimd.dma_start(
            out=wkv[:].rearrange("p (dc c) -> p dc c", c=D),
            in_=wkv_d[:].rearrange("(dc p) c -> p dc c", p=128))
        nc.gpsimd.dma_start(
            out=wq[:].rearrange("p (dc c) -> p dc c", c=D),
            in_=wq_d[:].rearrange("(dc p) c -> p dc c", p=128))

        es_rows = ExitStack()
        rowsp = es_rows.enter_context(tc.tile_pool(name="rows", bufs=1))
        embp = es_rows.enter_context(tc.tile_pool(name="embp", bufs=2))

        # rows: sng/snb via ONE dma; bias rows when present
        gsn = rowsp.tile([1, 2 * D], FP32R)
        nc.sync.dma_start(out=gsn[:], in_=gsn_d[:].rearrange("a b -> (a b)").unsqueeze(0))
        sng_row = gsn[:, 0:D]
        snb_row = gsn[:, D:2 * D]

        def load_row(pool, dram_ap, n):
            t_ = pool.tile([1, n], FP32R, tag=dram_ap.tensor.name)
            nc.sync.dma_start(out=t_[:], in_=dram_ap.unsqueeze(0))
            return t_

        cemb_row = load_row(constp, cemb_d.ap(), 2 * D) if has_cemb else None
        cq_row = load_row(constp, cq_d.ap(), D) if has_cq else None
        ck_row = load_row(constp, ck_d.ap(), D) if has_ck else None
        cv_row = load_row(constp, cv_d.ap(), D) if has_cv else None
        co_row = load_row(constp, co_d.ap(), D) if has_co else None

        # embt host-permuted: one DMA fills [128, 8], (p, c) = emb[c*128+p]
        embt_sb = rowsp.tile([128, ND], FP32)
        nc.sync.dma_start(out=embt_sb[:], in_=embt_d[:].rearrange(
            "(p c) -> p c", c=ND))

        # ---------------- stage 0+1: load x, LN (batched sqrt), kv ----------------
        # per-group LN stat tiles (group = 4 token tiles)
        aggs, rstds, nmrs = {}, {}, {}

        def em_load(t):
            xt = xio.tile([128, D], BF16, tag="xin")
            nc.sync.dma_start(out=xt[:], in_=x_d[t * 128:(t + 1) * 128, :])
            return xt

        xts = {}

        def em_stats(t):
            xt = xts[t]
            st6 = statp.tile([128, 2, 6], FP32, tag="st6")
            nc.vector.bn_stats(st6[:, 0, :], xt[:, 0:512])
            nc.vector.bn_stats(st6[:, 1, :], xt[:, 512:1024])
            g, i = t // 4, t % 4
            if i == 0:
                agg4 = statp.tile([128, 2, 4], FP32, tag="agg4")
                aggs[g] = agg4
            nc.vector.bn_aggr(aggs[g][:, :, i], st6[:])

        def em_group(g):
            # rstd = 1/sqrt(var+eps), nmr = -mean*rstd for 4 tiles at once
            agg = aggs.pop(g)
            rstd = statp.tile([128, 4], FP32, tag="rstd4")
            nc.vector.tensor_scalar(rstd[:], agg[:, 1, :], EPS, None,
                                    mybir.AluOpType.add)
            nc.scalar.sqrt(rstd[:], rstd[:])
            nc.vector.reciprocal(rstd[:], rstd[:])
            nmr = statp.tile([128, 4], FP32, tag="nmr4")
            nc.vector.scalar_tensor_tensor(nmr[:], agg[:, 0, :], -1.0,
                                           rstd[:], mybir.AluOpType.mult,
                                           mybir.AluOpType.mult)
            rstds[g], nmrs[g] = rstd, nmr

        def em_apply_tp(t):
            xt = xts.pop(t)
            g, i = t // 4, t % 4
            xn = xnp.tile([128, D], BF16, tag="xn")
            nc.scalar.activation(xn[:], xt[:],
                                 mybir.ActivationFunctionType.Identity,
                                 bias=nmrs[g][:, i:i + 1],
                                 scale=rstds[g][:, i:i + 1])
            for gg in range(2):  # groups of 4 d-chunks
                tpt = tp.tile([128, 512], BF16, tag="tp")
                for ii in range(4):
                    dc = gg * 4 + ii
                    nc.tensor.transpose(tpt[:, ii * 128:(ii + 1) * 128],
                                        xn[:, dc * 128:(dc + 1) * 128],
                                        ident_bf[:])
                dst = xnt[:].rearrange("p (dc tt) -> p dc tt", tt=TL)[
                    :, gg * 4:(gg + 1) * 4, t * 128:(t + 1) * 128]
                src_ = tpt[:].rearrange("p (i c) -> p i c", c=128)
                nc.vector.tensor_copy(dst, src_)

        ctx_sb = constp.tile([128, 8 * 65], FP32)

        def em_kv(t):
            ke = kvp.tile([128, D], BF16, tag="ke")
            va = kvp.tile([128, H * 66], BF16, tag="va")
            phs = []
            for j in range(4):  # kh0, kh1, vh0, vh1 accumulate together
                ph = pp.tile([128, 512], FP32, tag="pp", name=f"kvps{j}")
                phs.append(ph)
            for dc in range(ND):
                lhs = xnt[:, dc * TL + t * 128: dc * TL + (t + 1) * 128]
                for j, ph in enumerate(phs):
                    off = 0 if j < 2 else 8192
                    jh = j % 2
                    hb = has_ck if j < 2 else has_cv
                    nc.tensor.matmul(
                        ph[:], lhs,
                        wkv[:, off + dc * 1024 + jh * 512:off + dc * 1024 + (jh + 1) * 512],
                        start=(dc == 0), stop=(dc == 7 and not hb))
            if has_ck:
                for j in (0, 1):
                    nc.tensor.matmul(phs[j][:], ones_row[:, 0:128],
                                     ck_row[:, j * 512:(j + 1) * 512],
                                     start=False, stop=True)
            if has_cv:
                for j in (2, 3):
                    nc.tensor.matmul(phs[j][:], ones_row[:, 0:128],
                                     cv_row[:, (j - 2) * 512:(j - 1) * 512],
                                     start=False, stop=True)
            for jh in range(2):
                nc.scalar.activation(ke[:, jh * 512:(jh + 1) * 512],
                                     phs[jh][:],
                                     mybir.ActivationFunctionType.Exp)
            for jh in range(2):
                nc.vector.tensor_copy(
                    va[:].rearrange("p (h l) -> p h l", l=66)[
                        :, jh * 8:(jh + 1) * 8, 0:64],
                    phs[2 + jh][:].rearrange("p (h l) -> p h l", l=64))
            nc.vector.memset(
                va[:].rearrange("p (h l) -> p h l", l=66)[:, :, 64:65], 1.0)
            # one single-shot matmul per psum tile (bank sharing between
            # single-shot groups corrupts data on HW), then DVE-accumulate
            # into the SBUF ctx accumulator. Heads pack pairwise by parity.
            for hp in range(H // 2):
                cp = tp.tile([128, 512], FP32, tag="tp")
                for par in range(2):
                    h = 2 * hp + par
                    nc.tensor.matmul(cp[par * 64:par * 64 + 64, 0:65],
                                     ke[:, h * 64:(h + 1) * 64],
                                     va[:, h * 66:h * 66 + 65])
                if t == 0:
                    nc.vector.tensor_copy(
                        ctx_sb[:, hp * 65:(hp + 1) * 65], cp[:, 0:65])
                else:
                    nc.vector.tensor_tensor(
                        ctx_sb[:, hp * 65:(hp + 1) * 65],
                        ctx_sb[:, hp * 65:(hp + 1) * 65],
                        cp[:, 0:65], mybir.AluOpType.add)

        for tt in range(NT + 6):
            if tt < NT:
                xts[tt] = em_load(tt)
                em_stats(tt)
                if tt % 4 == 3:
                    em_group(tt // 4)
            if 4 <= tt < NT + 4:
                em_apply_tp(tt - 4)
            if tt >= 6:
                em_kv(tt - 6)

        # ---------------- emb MLP (emitted after kv: PE queue stays dense) ----------------
        silu_e = rowsp.tile([128, ND], FP32)
        if USE_NATIVE_SILU:
            nc.scalar.activation(silu_e[:], embt_sb[:],
                                 mybir.ActivationFunctionType.Silu)
        else:
            nc.scalar.activation(silu_e[:], embt_sb[:],
                                 mybir.ActivationFunctionType.Sigmoid)
            nc.vector.tensor_tensor(silu_e[:], silu_e[:], embt_sb[:],
                                    mybir.AluOpType.mult)
        silu_eb = rowsp.tile([128, 2 * ND], BF16)
        nc.vector.tensor_copy(
            silu_eb[:].rearrange("p (c two) -> p c two", two=2)[:, :, 0:1],
            silu_e[:].unsqueeze(2))
        emb_sel = rowsp.tile([1, 2 * D], FP32R)
        for dc in range(ND):
            embw_ch = embp.tile([128, 2 * D], BF16, tag="embw_ch")
            nc.gpsimd.dma_start(out=embw_ch[:],
                                in_=embw_d[dc * 128:(dc + 1) * 128, :])
            for nch in range(4):
                epn = pp.tile([1, 512], FP32, tag="pp")
                nc.tensor.matmul(epn[:],
                                 silu_eb[:, 2 * dc:2 * dc + 1],
                                 embw_ch[:, nch * 512:(nch + 1) * 512])
                dst = emb_sel[:, nch * 512:(nch + 1) * 512]
                if dc == 0:
                    nc.vector.tensor_copy(dst, epn[:])
                else:
                    nc.vector.tensor_tensor(dst, dst, epn[:],
                                            mybir.AluOpType.add)
        if has_cemb:
            nc.vector.tensor_tensor(emb_sel[:], emb_sel[:], cemb_row[:],
                                    mybir.AluOpType.add)
        # broadcast emb_sel + sng/snb rows to all partitions
        emb_sel_b = rowsp.tile([128, 2 * D], FP32)
        for nch in range(4):
            bp = tp.tile([128, 512], FP32, tag="tp")
            nc.tensor.matmul(bp[:], ones_row[:, 0:128],
                             emb_sel[:, nch * 512:(nch + 1) * 512])
            nc.vector.tensor_copy(emb_sel_b[:, nch * 512:(nch + 1) * 512], bp[:])

        def bcast(row, name):
            out = rowsp.tile([128, D], FP32, tag=f"bc_{name}")
            for nh in range(2):
                bp = tp.tile([128, 512], FP32, tag="tp")
                nc.tensor.matmul(bp[:], ones_row[:, 0:128],
                                 row[:, nh * 512:(nh + 1) * 512])
                nc.vector.tensor_copy(out[:, nh * 512:(nh + 1) * 512], bp[:])
            return out

        sng_b = bcast(sng_row, "sng")
        snb_b = bcast(snb_row, "snb")
        # scale2 = sng*(1+scale); shift2 = snb*(1+scale) + shift
        t1_b = rowsp.tile([128, D], FP32)
        nc.vector.tensor_scalar(t1_b[:], emb_sel_b[:, 0:D], 1.0, None,
                                mybir.AluOpType.add)
        s2_b = constp.tile([128, D], FP32)
        nc.vector.tensor_tensor(s2_b[:], t1_b[:], sng_b[:],
                                mybir.AluOpType.mult)
        sh2_b = constp.tile([128, D], FP32)
        nc.vector.tensor_tensor(sh2_b[:], t1_b[:], snb_b[:],
                                mybir.AluOpType.mult)
        nc.vector.tensor_tensor(sh2_b[:], sh2_b[:], emb_sel_b[:, D:2 * D],
                                mybir.AluOpType.add)
        es_rows.close()

        # ---------------- stage 3a: stage ctx + pairwise AllReduce ----------------
        cc_in = dramp.tile([H, 64, 65], FP32)
        cc_out = dramp.tile([H, 64, 65], FP32)
        for q in range(2):
            nc.sync.dma_start(
                out=cc_in[:].rearrange("(g q) d l -> q d g l", q=2)[q],
                in_=ctx_sb[q * 64:(q + 1) * 64, :].rearrange(
                    "d (g l) -> d g l", l=65))
        if USE_COLLECTIVE:
            nc.gpsimd.collective_compute(
                "AllReduce", mybir.AluOpType.add,
                replica_groups=[[0, 1], [2, 3], [4, 5], [6, 7]],
                ins=[cc_in.opt()], outs=[cc_out.opt()])
        else:
            nc.sync.dma_start(out=cc_out[:], in_=cc_in[:])
        es_wkv.close()

        # ---------------- stage 2: q proj + exp (overlaps collective) ----------------
        es_qt = ExitStack()
        qtp = es_qt.enter_context(tc.tile_pool(name="qT", bufs=1, side="right"))
        es_out = ExitStack()
        wop = es_out.enter_context(tc.tile_pool(name="wo", bufs=1, side="right"))
        ytp = es_out.enter_context(tc.tile_pool(name="ytp", bufs=5, side="right"))
        lnp = es_out.enter_context(tc.tile_pool(name="lnp", bufs=2, side="right"))
        hsp = es_out.enter_context(tc.tile_pool(name="hsp", bufs=3, side="right"))
        wo = wop.tile([128, ND * 1024], BF16)
        nc.gpsimd.dma_start(
            out=wo[:].rearrange("p (dc c) -> p dc c", c=D),
            in_=wo_d[:].rearrange("(dc p) c -> p dc c", p=128))
        qT = qtp.tile([128, ND * TL], BF16)  # j-chunk jc at cols jc*TL
        for jc in range(ND):
            qpss = []
            for th in range(TL // 512):
                qps = pp.tile([128, 512], FP32, tag="pp", name=f"qps{th}")
                qpss.append(qps)
            for dc in range(ND):
                w_sl = wq[:, dc * 1024 + jc * 128:dc * 1024 + jc * 128 + 128]
                for th in range(TL // 512):
                    nc.tensor.matmul(
                        qpss[th][:], w_sl,
                        xnt[:, dc * TL + th * 512:dc * TL + (th + 1) * 512],
                        start=(dc == 0), stop=(dc == 7 and not has_cq))
            if has_cq:
                for th in range(TL // 512):
                    nc.tensor.matmul(qpss[th][:],
                                     cq_row[:, jc * 128:(jc + 1) * 128],
                                     ones_row[:], start=False, stop=True)
            for th in range(TL // 512):
                nc.scalar.activation(
                    qT[:, jc * TL + th * 512:jc * TL + (th + 1) * 512],
                    qpss[th][:], mybir.ActivationFunctionType.Exp)
        es_wq.close()
        es_xnt.close()

        # ---------------- stage 4: readback + normalize ctx into block-diag ----------------
        # ctx_bd layout per j-chunk jc (130 bf16 cols at jc*130):
        #   cols 0:64    = ctx_hat for head 2jc   on partitions 0:64, zeros below
        #   cols 64:128  = ctx_hat for head 2jc+1 on partitions 64:128, zeros above
        #   col 128      = ones on partitions 0:64   (-> qden of head 2jc)
        #   col 129      = ones on partitions 64:128 (-> qden of head 2jc+1)
        ctxn = constp.tile([128, 8 * 65], FP32)
        for q in range(2):
            nc.sync.dma_start(
                out=ctxn[q * 64:(q + 1) * 64, :].rearrange(
                    "d (g l) -> d g l", l=65),
                in_=cc_out[:].rearrange("(g q) d l -> q d g l", q=2)[q])
        rk = statp.tile([128, 8], FP32, tag="rk")
        nc.vector.reciprocal(
            rk[:], ctxn[:].rearrange("p (g l) -> p g l", l=65)[:, :, 64])
        ctx_bd = constp.tile([128, 8 * 130], BF16)
        nc.vector.memset(ctx_bd[:], 0.0)
        for g in range(8):
            for par in range(2):
                nc.vector.tensor_scalar(
                    ctx_bd[par * 64:(par + 1) * 64,
                           g * 130 + par * 64:g * 130 + par * 64 + 64],
                    ctxn[par * 64:(par + 1) * 64, g * 65:g * 65 + 64],
                    rk[par * 64:(par + 1) * 64, g:g + 1], None,
                    mybir.AluOpType.mult)
            nc.vector.memset(ctx_bd[0:64, g * 130 + 128:g * 130 + 129], 1.0)
            nc.vector.memset(ctx_bd[64:128, g * 130 + 129:g * 130 + 130], 1.0)

        # ---------------- stage 5+6 fused per token tile ----------------
        yts, hss = {}, {}
        agg2s, rstd2s, nmr2s = {}, {}, {}

        def em_y(t):
            yt = ytp.tile([128, D], BF16, tag="yt")
            yts[t] = yt
            for jc in range(ND):
                ypf = pp.tile([128, 512], FP32, tag="pp")
                yp = ypf[:, 0:130]
                nc.tensor.matmul(
                    yp[:],
                    qT[:, jc * TL + t * 128:jc * TL + (t + 1) * 128],
                    ctx_bd[:, jc * 130:(jc + 1) * 130])
                rq = statp.tile([128, 2], FP32, tag="rq")
                nc.vector.reciprocal(rq[:], yp[:, 128:130])
                # y[:, jc*128:(jc+1)*128] = yp[:, 0:128] * qden^-1 per head
                nc.vector.tensor_tensor(
                    yt[:, jc * 128:(jc + 1) * 128].rearrange(
                        "p (h l) -> p h l", l=64),
                    yp[:, 0:128].rearrange("p (h l) -> p h l", l=64),
                    rq[:].unsqueeze(2).broadcast_to([128, 2, 64]),
                    mybir.AluOpType.mult)

        def em_stats2(t):
            yt = yts[t]
            st6 = statp.tile([128, 2, 6], FP32, tag="st6")
            nc.vector.bn_stats(st6[:, 0, :], yt[:, 0:512])
            nc.vector.bn_stats(st6[:, 1, :], yt[:, 512:1024])
            g, i = t // 4, t % 4
            if i == 0:
                agg4b = statp.tile([128, 2, 4], FP32, tag="agg4b")
                agg2s[g] = agg4b
            nc.vector.bn_aggr(agg2s[g][:, :, i], st6[:])

        def em_group2(g):
            agg = agg2s.pop(g)
            rstd = statp.tile([128, 4], FP32, tag="rstd4b")
            nc.vector.tensor_scalar(rstd[:], agg[:, 1, :], EPS, None,
                                    mybir.AluOpType.add)
            nc.scalar.sqrt(rstd[:], rstd[:])
            nc.vector.reciprocal(rstd[:], rstd[:])
            nmr = statp.tile([128, 4], FP32, tag="nmr4b")
            nc.vector.scalar_tensor_tensor(nmr[:], agg[:, 0, :], -1.0,
                                           rstd[:], mybir.AluOpType.mult,
                                           mybir.AluOpType.mult)
            rstd2s[g], nmr2s[g] = rstd, nmr

        def em_ln(t):
            yt = yts.pop(t)
            g, i = t // 4, t % 4
            ln = lnp.tile([128, D], BF16, tag="ln")
            nc.scalar.activation(ln[:], yt[:],
                                 mybir.ActivationFunctionType.Identity,
                                 bias=nmr2s[g][:, i:i + 1],
                                 scale=rstd2s[g][:, i:i + 1])
            nc.vector.tensor_tensor(ln[:], ln[:], s2_b[:],
                                    mybir.AluOpType.mult)
            nc.gpsimd.tensor_tensor(ln[:], ln[:], sh2_b[:],
                                    mybir.AluOpType.add)
            hs = hsp.tile([128, D], BF16, tag="hs")
            if USE_NATIVE_SILU:
                nc.scalar.activation(hs[:], ln[:],
                                     mybir.ActivationFunctionType.Silu)
            else:
                nc.scalar.activation(hs[:], ln[:],
                                     mybir.ActivationFunctionType.Sigmoid)
                nc.vector.tensor_tensor(hs[:], hs[:], ln[:],
                                        mybir.AluOpType.mult)
            hss[t] = hs

        def em_out(t):
            hs = hss.pop(t)
            hst = hsp.tile([128, D], BF16, tag="hst")
            for g in range(2):
                tpt = tp.tile([128, 512], BF16, tag="tp")
                for i in range(4):
                    dc = g * 4 + i
                    nc.tensor.transpose(tpt[:, i * 128:(i + 1) * 128],
                                        hs[:, dc * 128:(dc + 1) * 128],
                                        ident_bf[:])
                nc.vector.tensor_copy(hst[:, g * 512:(g + 1) * 512], tpt[:])
            xt2 = xio.tile([128, D], BF16, tag="xin")
            nc.gpsimd.dma_start(out=xt2[:], in_=x_d[t * 128:(t + 1) * 128, :])
            fin = finp.tile([128, D], FP32, tag="fin")
            ophs = []
            for jh in range(2):
                oph = pp.tile([128, 512], FP32, tag="pp", name=f"oph{jh}")
                ophs.append(oph)
            for dc in range(ND):
                h_sl = hst[:, dc * 128:(dc + 1) * 128]
                for jh in range(2):
                    nc.tensor.matmul(
                        ophs[jh][:], h_sl,
                        wo[:, dc * 1024 + jh * 512:dc * 1024 + (jh + 1) * 512],
                        start=(dc == 0), stop=(dc == 7 and not has_co))
            if has_co:
                for jh in range(2):
                    nc.tensor.matmul(ophs[jh][:], ones_row[:, 0:128],
                                     co_row[:, jh * 512:(jh + 1) * 512],
                                     start=False, stop=True)
            for jh in range(2):
                nc.vector.tensor_tensor(fin[:, jh * 512:(jh + 1) * 512],
                                        ophs[jh][:],
                                        xt2[:, jh * 512:(jh + 1) * 512],
                                        mybir.AluOpType.add)
            nc.sync.dma_start(out=out_d[t * 128:(t + 1) * 128, :], in_=fin[:])

        for tt in range(NT + 6):
            if tt < NT:
                em_y(tt)
                em_stats2(tt)
                if tt % 4 == 3:
                    em_group2(tt // 4)
            if 4 <= tt < NT + 4:
                em_ln(tt - 4)
            if tt >= 6:
                em_out(tt - 6)
        es_out.close()
        es_qt.close()

    with tile.TileContext(nc) as tc, ExitStack() as es:
        _emit(tc, es)
    nc.compile()
    _legalize_waits(nc)
    return nc


def kernel(**inputs):
    x = np.asarray(inputs["x"], np.float32)
    emb = np.asarray(inputs["emb"], np.float32)
    gate_msa = np.asarray(inputs["gate_msa"], np.float32)
    norm_g = np.asarray(inputs["norm_g"], np.float32)
    norm_b = np.asarray(inputs["norm_b"], np.float32)
    Wq = np.asarray(inputs["Wq"], np.float32)
    bq = np.asarray(inputs["bq"], np.float32)
    Wk = np.asarray(inputs["Wk"], np.float32)
    bk = np.asarray(inputs["bk"], np.float32)
    Wv = np.asarray(inputs["Wv"], np.float32)
    bv = np.asarray(inputs["bv"], np.float32)
    emb_W = np.asarray(inputs["emb_W"], np.float32)
    emb_b = np.asarray(inputs["emb_b"], np.float32)
    sn_g = np.asarray(inputs["sn_g"], np.float32)
    sn_b = np.asarray(inputs["sn_b"], np.float32)
    out_W = np.asarray(inputs["out_W"], np.float32)
    out_b = np.asarray(inputs["out_b"], np.float32)

    import ml_dtypes

    # fold layernorm affine into projection weights
    wq_f = norm_g[:, None] * Wq
    wk_f = norm_g[:, None] * Wk
    wv_f = norm_g[:, None] * Wv
    wkv_f = np.ascontiguousarray(
        np.concatenate([wk_f, wv_f], axis=0).astype(ml_dtypes.bfloat16))
    wq_f = wq_f.astype(ml_dtypes.bfloat16)
    embw_bf = emb_W.astype(ml_dtypes.bfloat16)
    cq = norm_b @ Wq + bq
    ck = norm_b @ Wk + bk
    cv = norm_b @ Wv + bv

    # gate folded into out_W per batch: out = x + (h @ (out_W*gate)) + gate*out_b
    wo_gated = [np.ascontiguousarray(
        (out_W * gate_msa[b, 0, :][None, :]).astype(ml_dtypes.bfloat16))
        for b in range(B)]
    co_gated = [gate_msa[b, 0, :] * out_b for b in range(B)]

    flags = (bool(np.any(cq)), bool(np.any(ck)), bool(np.any(cv)),
             bool(np.any(out_b)), bool(np.any(emb_b)))
    if flags not in _CACHE:
        _CACHE[flags] = build(*flags)
    nc = _CACHE[flags]

    in_maps = []
    for c in range(NCORES):
        b, half = c // 2, c % 2
        m = {
            "x": np.ascontiguousarray(
                x[b, half * TL:(half + 1) * TL, :].astype(ml_dtypes.bfloat16)),
            "wq": wq_f, "wkv": wkv_f, "wo": wo_gated[b],
            "embw": embw_bf,
            "embt": np.ascontiguousarray(emb[b, 0, :].reshape(ND, 128).T.ravel()),
            "gsn": np.ascontiguousarray(np.stack([sn_g, sn_b])),
        }
        if flags[0]:
            m["cq"] = cq
        if flags[1]:
            m["ck"] = ck
        if flags[2]:
            m["cv"] = cv
        if flags[3]:
            m["co"] = co_gated[b]
        if flags[4]:
            m["cemb"] = emb_b
        in_maps.append(m)

    res = run_bass_kernel_spmd(nc, in_maps, core_ids=list(range(NCORES)),
                               **_RUN_KW)
    kernel.last_result = res
    out = np.stack([res.results[c]["out"] for c in range(NCORES)])
    return out.reshape(B, 2, TL, D).reshape(B, T, D)


_RUN_KW = {}
kernel.last_result = None
